# revision 13
# baseline (speedup 1.0000x reference)
"""Multi-head attention (B=8, S=1024, E=768, H=12, D=64) on 8 TRN2 NeuronCores.

Strategy: pure data-parallel over batch — each core processes one batch
element end-to-end (all 12 heads), so no collectives are needed. Inside a
core everything is kept "transposed" so the PE contraction dim always sits
on SBUF partitions:

  host:   xT = x[b].T                              [E, S]
  qT/kT:  out[d_pair, S]   = Wpair.T @ xT          (Wq slices stationary)
  v:      out[t, h*D]      = xT_chunk.T @ Wv_pack  (+ bias via ones-row K=1 matmul)
  scores: out[t_chunk, S]  = kT_slice.T @ qT       (K=64, two heads row-packed)
  P = exp(scale*scores)    on ACT, no max-subtraction (scores are O(1))
  AV:     out[65, S]       = [v_h | 1].T @ P       row 64 = softmax denominator
  norm:   attnT = AV[0:64] * broadcast(1/AV[64])   (DMA-broadcast + DVE mult)
  y:      y[s_chunk, E]    = attnT_pair.T @ Wo_chunk (+ bo via ones-row)
"""

import sys
import types

import numpy as np

import concourse.bacc as bacc
import concourse.bass as bass
import concourse.mybir as mybir
import concourse.tile as tile
from concourse.bass_utils import run_bass_kernel_spmd


def _ensure_ntff_hook():
    """This image's antenv lacks axon_hooks; synthesize it so trace=True works."""
    try:
        import antenv.axon_hooks  # noqa: F401

        return
    except ImportError:
        pass
    try:
        from trn_agent_boot.trn_boot import _ntff_profile_via_ctypes

        hook = _ntff_profile_via_ctypes("/opt/axon/libaxon_pjrt.so")
    except Exception:
        hook = None
    m = types.ModuleType("antenv.axon_hooks")
    m.get_axon_ntff_profile_hook = lambda: hook
    m.set_axon_ntff_profile_hook = lambda h: None
    sys.modules["antenv.axon_hooks"] = m


_ensure_ntff_hook()

F32 = mybir.dt.float32
P = 128
S = 1024
E = 768
H = 12
D = 64
NCORES = 8

NE = E // P    # 6 contraction chunks over E
NPAIR = H // 2 # 6 head pairs
NT = S // P    # 8 key/t chunks
NJ = 2         # 512-wide free-dim chunks over S
JW = 512
SCALE = 1.0 / 8.0  # 1/sqrt(D)

EXP_BUFS = 14


def build_nc():
    nc = bacc.Bacc(
        "TRN2",
        target_bir_lowering=False,
        debug=False,
        num_devices=NCORES,
    )
    xt_d = nc.declare_dram_parameter("xt", [E, S], F32, isOutput=False)
    wq_d = nc.declare_dram_parameter("wq", [E, E], F32, isOutput=False)
    wk_d = nc.declare_dram_parameter("wk", [E, E], F32, isOutput=False)
    wv_d = nc.declare_dram_parameter("wv", [E, E], F32, isOutput=False)
    bq_d = nc.declare_dram_parameter("bq", [E], F32, isOutput=False)
    bk_d = nc.declare_dram_parameter("bk", [E], F32, isOutput=False)
    bv_d = nc.declare_dram_parameter("bv", [E], F32, isOutput=False)
    wo_d = nc.declare_dram_parameter("wo", [E, E], F32, isOutput=False)
    bo_d = nc.declare_dram_parameter("bo", [E], F32, isOutput=False)
    y_d = nc.declare_dram_parameter("y", [S, E], F32, isOutput=True)

    with tile.TileContext(nc) as tc:
        with (
            tc.tile_pool(name="persist", bufs=1) as persist,
            tc.tile_pool(name="work", bufs=2) as work,
        ):
            # ---- persistent SBUF tensors (one producer per tile) ----
            qt = [persist.tile([P, S], F32, name=f"qt{i}", tag=f"qt{i}") for i in range(NPAIR)]
            kt = [persist.tile([P, S], F32, name=f"kt{i}", tag=f"kt{i}") for i in range(NPAIR)]
            vx = [
                persist.tile([P, H * (D + 1)], F32, name=f"vx{c}", tag=f"vx{c}") for c in range(NT)
            ]
            at = [persist.tile([P, S], F32, name=f"at{i}", tag=f"at{i}") for i in range(NPAIR)]
            wo_s = [persist.tile([P, E], F32, name=f"wo{i}", tag=f"wo{i}") for i in range(NE)]
            bq_s = persist.tile([P, NPAIR], F32)
            bk_s = persist.tile([P, NPAIR], F32)
            bv_r = persist.tile([1, E], F32)
            bo_r = persist.tile([1, E], F32)
            ones = persist.tile([1, P], F32)

            nc.vector.memset(ones[:, :], 1.0)

            # ---- bias / weight loads ----
            nc.sync.dma_start(bq_s[:, :], bq_d[:].rearrange("(c p) -> p c", p=P))
            nc.sync.dma_start(bk_s[:, :], bk_d[:].rearrange("(c p) -> p c", p=P))
            nc.sync.dma_start(bv_r[:, :], bv_d[:].unsqueeze(0))
            nc.sync.dma_start(bo_r[:, :], bo_d[:].unsqueeze(0))
            wo_r = wo_d.rearrange("(i p) e -> p i e", p=P)
            for i in range(NE):
                nc.sync.dma_start(wo_s[i][:, :], wo_r[:, i, :])

            with (
                tc.tile_pool(name="loads", bufs=1) as loads,
                tc.tile_pool(name="wqk_stream", bufs=14) as wqk_stream,
                tc.tile_pool(name="ps_qkv", bufs=2, space="PSUM") as ps_qkv,
            ):
                xt = [loads.tile([P, S], F32, name=f"xt{i}", tag=f"xt{i}") for i in range(NE)]
                wv_s = [loads.tile([P, E], F32, name=f"wv{i}", tag=f"wv{i}") for i in range(NE)]
                xt_r = xt_d.rearrange("(i p) s -> p i s", p=P)
                wq_r = wq_d.rearrange("(i p) e -> p i e", p=P)
                wk_r = wk_d.rearrange("(i p) e -> p i e", p=P)
                wv_r = wv_d.rearrange("(i p) e -> p i e", p=P)
                for i in range(NE):
                    nc.sync.dma_start(xt[i][:, :], xt_r[:, i, :])
                    nc.sync.dma_start(wv_s[i][:, :], wv_r[:, i, :])

                # ---- qT / kT: per pair, stationary = W slice, moving = xT ----
                for pr in range(NPAIR):
                    for w_r, b_s, dst in ((wq_r, bq_s, qt), (wk_r, bk_s, kt)):
                        wts = []
                        for i in range(NE):
                            wt = wqk_stream.tile([P, P], F32, tag="wqk")
                            nc.sync.dma_start(wt[:, :], w_r[:, i, bass.ts(pr, P)])
                            wts.append(wt)
                        ps = ps_qkv.tile([P, S], F32, tag="ps_qk")
                        for j in range(NJ):
                            jsl = bass.ts(j, JW)
                            for i in range(NE):
                                nc.tensor.matmul(
                                    ps[:, jsl],
                                    wts[i][:, :],
                                    xt[i][:, jsl],
                                    start=(i == 0),
                                    stop=(i == NE - 1),
                                )
                        nc.vector.tensor_scalar_add(
                            dst[pr][:, :], ps[:, :], b_s[:, pr : pr + 1]
                        )

                # ---- v (natural layout): stationary = xT chunk, moving = Wv ----
                for c in range(NT):
                    ps = ps_qkv.tile([P, E], F32, tag="ps_v")
                    for n0, nw in ((0, JW), (JW, E - JW)):
                        nsl = bass.ds(n0, nw)
                        for i in range(NE):
                            nc.tensor.matmul(
                                ps[:, nsl],
                                xt[i][:, bass.ts(c, P)],
                                wv_s[i][:, nsl],
                                start=(i == 0),
                                stop=False,
                            )
                        # bias via rank-1 update: ones.T @ bv_row
                        nc.tensor.matmul(
                            ps[:, nsl],
                            ones[:, :],
                            bv_r[:, nsl],
                            start=False,
                            stop=True,
                        )
                    vx4 = vx[c][:, :].rearrange("p (h e) -> p h e", e=D + 1)
                    nc.vector.tensor_copy(
                        vx4[:, :, 0:D],
                        ps[:, :].rearrange("p (h e) -> p h e", e=D),
                    )
                    nc.vector.memset(vx4[:, :, D], 1.0)

            # ---- attention ----
            with tc.tile_pool(name="exp_pool", bufs=EXP_BUFS) as exp_pool:
                with (
                    tc.tile_pool(name="ps_s", bufs=2, space="PSUM") as ps_s_pool,
                    tc.tile_pool(name="ps_av", bufs=2, space="PSUM") as ps_av_pool,
                ):
                    for pr in range(NPAIR):
                        exps = [[None] * NT for _ in range(2)]
                        for r in range(2):
                            rsl = bass.ds(64 * r, 64)
                            for c in range(NT):
                                et = exp_pool.tile([P, S], F32, tag="exp")
                                for j in range(NJ):
                                    jsl = bass.ts(j, JW)
                                    ps = ps_s_pool.tile([P, JW], F32, tag="ps_s")
                                    nc.tensor.matmul(
                                        ps[:, :],
                                        kt[pr][rsl, bass.ts(c, P)],
                                        qt[pr][rsl, jsl],
                                        start=True,
                                        stop=True,
                                        tile_position=(64 * r, 0),
                                    )
                                    nc.scalar.activation(
                                        et[:, jsl],
                                        ps[:, :],
                                        mybir.ActivationFunctionType.Exp,
                                        scale=SCALE,
                                    )
                                exps[r][c] = et
                        for r in range(2):
                            h = 2 * pr + r
                            hsl = bass.ds(h * (D + 1), D + 1)
                            av = ps_av_pool.tile([P, S], F32, tag="av")
                            for c in range(NT):
                                for j in range(NJ):
                                    jsl = bass.ts(j, JW)
                                    nc.tensor.matmul(
                                        av[0 : D + 1, jsl],
                                        vx[c][:, hsl],
                                        exps[r][c][:, jsl],
                                        start=(c == 0),
                                        stop=(c == NT - 1),
                                    )
                            rec = work.tile([1, S], F32, tag="rec")
                            nc.vector.reciprocal(rec[:, :], av[D : D + 1, :])
                            bc = work.tile([D, S], F32, tag="bc")
                            nc.sync.dma_start(
                                bc[:, :],
                                rec[0:1, :].unsqueeze(1).to_broadcast((1, D, S)),
                            )
                            nc.vector.tensor_tensor(
                                at[pr][bass.ds(64 * r, 64), :],
                                av[0:D, :],
                                bc[:, :],
                                mybir.AluOpType.mult,
                            )

            # ---- output projection ----
            with tc.tile_pool(name="ps_y", bufs=2, space="PSUM") as ps_y_pool:
                for j in range(NT):
                    ps = ps_y_pool.tile([P, E], F32, tag="ps_y")
                    for n0, nw in ((0, JW), (JW, E - JW)):
                        nsl = bass.ds(n0, nw)
                        for pr in range(NPAIR):
                            nc.tensor.matmul(
                                ps[:, nsl],
                                at[pr][:, bass.ts(j, P)],
                                wo_s[pr][:, nsl],
                                start=(pr == 0),
                                stop=False,
                            )
                        nc.tensor.matmul(
                            ps[:, nsl], ones[:, :], bo_r[:, nsl], start=False, stop=True
                        )
                    ysb = work.tile([P, E], F32, tag="ysb")
                    nc.vector.tensor_copy(ysb[:, :], ps[:, :])
                    nc.sync.dma_start(y_d[bass.ts(j, P), :], ysb[:, :])

    nc.compile()
    return nc


_NC = None


def _get_nc():
    global _NC
    if _NC is None:
        _NC = build_nc()
    return _NC


def _prep_inputs(hidden_state, Wq, bq, Wk, bk, Wv, bv, Wo, bo):
    """Build the per-core input maps (data-parallel over batch)."""
    f = np.float32
    wq_p = np.ascontiguousarray(Wq.transpose(1, 0, 2).reshape(E, E), dtype=f)
    wk_p = np.ascontiguousarray(Wk.transpose(1, 0, 2).reshape(E, E), dtype=f)
    wv_p = np.ascontiguousarray(Wv.transpose(1, 0, 2).reshape(E, E), dtype=f)
    bq_p = np.ascontiguousarray(bq.reshape(E), dtype=f)
    bk_p = np.ascontiguousarray(bk.reshape(E), dtype=f)
    bv_p = np.ascontiguousarray(bv.reshape(E), dtype=f)
    wo_p = np.ascontiguousarray(Wo, dtype=f)
    bo_p = np.ascontiguousarray(bo, dtype=f)
    in_maps = []
    for b in range(NCORES):
        in_maps.append(
            {
                "xt": np.ascontiguousarray(hidden_state[b].T, dtype=f),
                "wq": wq_p,
                "wk": wk_p,
                "wv": wv_p,
                "bq": bq_p,
                "bk": bk_p,
                "bv": bv_p,
                "wo": wo_p,
                "bo": bo_p,
            }
        )
    return in_maps


def kernel(hidden_state, Wq, bq, Wk, bk, Wv, bv, Wo, bo, _trace=False):
    nc = _get_nc()
    in_maps = _prep_inputs(hidden_state, Wq, bq, Wk, bk, Wv, bv, Wo, bo)
    res = run_bass_kernel_spmd(nc, in_maps, list(range(NCORES)), trace=_trace)
    out = np.stack([np.asarray(res.results[b]["y"]) for b in range(NCORES)])
    if _trace:
        kernel.last_exec_time_ns = res.exec_time_ns
        kernel.last_res = res
    return out.astype(np.float32)


# revision 20
# speedup vs baseline: 1.4573x; 1.4573x over previous
"""Multi-head attention (B=8, S=1024, E=768, H=12, D=64) on 8 TRN2 NeuronCores.

Strategy: pure data-parallel over batch — each core processes one batch
element end-to-end (all 12 heads), so no collectives are needed. Inside a
core everything is kept "transposed" so the PE contraction dim always sits
on SBUF partitions:

  host:   xT = x[b].T                              [E, S]
  qT/kT:  out[d_pair, S]   = Wpair.T @ xT          (Wq slices stationary)
  v:      out[t, h*D]      = xT_chunk.T @ Wv_pack  (+ bias via ones-row K=1 matmul)
  scores: out[t_chunk, S]  = kT_slice.T @ qT       (K=64, two heads row-packed)
  P = exp(scale*scores)    on ACT, no max-subtraction (scores are O(1))
  AV:     out[65, S]       = [v_h | 1].T @ P       row 64 = softmax denominator
  norm:   attnT = AV[0:64] * broadcast(1/AV[64])   (DMA-broadcast + DVE mult)
  y:      y[s_chunk, E]    = attnT_pair.T @ Wo_chunk (+ bo via ones-row)
"""

import sys
import types

import numpy as np

import concourse.bacc as bacc
import concourse.bass as bass
import concourse.mybir as mybir
import concourse.tile as tile
from concourse.bass_utils import run_bass_kernel_spmd


def _ensure_ntff_hook():
    """This image's antenv lacks axon_hooks; synthesize it so trace=True works."""
    try:
        import antenv.axon_hooks  # noqa: F401

        return
    except ImportError:
        pass
    try:
        from trn_agent_boot.trn_boot import _ntff_profile_via_ctypes

        hook = _ntff_profile_via_ctypes("/opt/axon/libaxon_pjrt.so")
    except Exception:
        hook = None
    m = types.ModuleType("antenv.axon_hooks")
    m.get_axon_ntff_profile_hook = lambda: hook
    m.set_axon_ntff_profile_hook = lambda h: None
    sys.modules["antenv.axon_hooks"] = m


_ensure_ntff_hook()

F32 = mybir.dt.float32
P = 128
S = 1024
E = 768
H = 12
D = 64
NCORES = 8

NE = E // P    # 6 contraction chunks over E
NPAIR = H // 2 # 6 head pairs
NT = S // P    # 8 key/t chunks
NJ = 2         # 512-wide free-dim chunks over S
JW = 512
SCALE = 1.0 / 8.0  # 1/sqrt(D)

EXP_BUFS = 14

F32R = mybir.dt.float32r


def _round_fp32r(a):
    """Host-side round-to-nearest of fp32 to the PE's fp32r format (9 mantissa bits)."""
    v = np.ascontiguousarray(a, dtype=np.float32).view(np.uint32)
    r = (v.astype(np.uint64) + 0x7FF + ((v >> 12) & 1)) & 0xFFFFF000
    return r.astype(np.uint32).view(np.float32)


def build_nc():
    nc = bacc.Bacc(
        "TRN2",
        target_bir_lowering=False,
        debug=False,
        num_devices=NCORES,
    )
    xt_d = nc.declare_dram_parameter("xt", [E, S], F32R, isOutput=False)
    wq_d = nc.declare_dram_parameter("wq", [E, E], F32R, isOutput=False)
    wk_d = nc.declare_dram_parameter("wk", [E, E], F32R, isOutput=False)
    wv_d = nc.declare_dram_parameter("wv", [E, E], F32R, isOutput=False)
    bq_d = nc.declare_dram_parameter("bq", [E], F32, isOutput=False)
    bk_d = nc.declare_dram_parameter("bk", [E], F32, isOutput=False)
    bv_d = nc.declare_dram_parameter("bv", [E], F32R, isOutput=False)
    wo_d = nc.declare_dram_parameter("wo", [E, E], F32R, isOutput=False)
    bo_d = nc.declare_dram_parameter("bo", [E], F32R, isOutput=False)
    ones_d = nc.declare_dram_parameter("ones", [P], F32R, isOutput=False)
    y_d = nc.declare_dram_parameter("y", [S, E], F32, isOutput=True)

    with tile.TileContext(nc) as tc:
        with (
            tc.tile_pool(name="persist", bufs=1) as persist,
            tc.tile_pool(name="work", bufs=2) as work,
        ):
            # ---- persistent SBUF tensors (one producer per tile) ----
            qt = [persist.tile([P, S], F32R, name=f"qt{i}", tag=f"qt{i}") for i in range(NPAIR)]
            kt = [persist.tile([P, S], F32R, name=f"kt{i}", tag=f"kt{i}") for i in range(NPAIR)]
            vx = [
                persist.tile([P, H * (D + 1)], F32R, name=f"vx{c}", tag=f"vx{c}") for c in range(NT)
            ]
            at = [persist.tile([P, S], F32R, name=f"at{i}", tag=f"at{i}") for i in range(NPAIR)]
            wo_s = [persist.tile([P, E], F32R, name=f"wo{i}", tag=f"wo{i}") for i in range(NE)]
            bq_s = persist.tile([P, NPAIR], F32)
            bk_s = persist.tile([P, NPAIR], F32)
            bv_r = persist.tile([1, E], F32R)
            bo_r = persist.tile([1, E], F32R)
            ones = persist.tile([1, P], F32R)

            nc.sync.dma_start(ones[:, :], ones_d[:].unsqueeze(0))

            # ---- bias / weight loads ----
            nc.sync.dma_start(bq_s[:, :], bq_d[:].rearrange("(c p) -> p c", p=P))
            nc.sync.dma_start(bk_s[:, :], bk_d[:].rearrange("(c p) -> p c", p=P))
            nc.sync.dma_start(bv_r[:, :], bv_d[:].unsqueeze(0))
            nc.sync.dma_start(bo_r[:, :], bo_d[:].unsqueeze(0))
            wo_r = wo_d.rearrange("(i p) e -> p i e", p=P)
            for i in range(NE):
                nc.sync.dma_start(wo_s[i][:, :], wo_r[:, i, :])

            with (
                tc.tile_pool(name="loads", bufs=1) as loads,
                tc.tile_pool(name="wqk_stream", bufs=14) as wqk_stream,
                tc.tile_pool(name="ps_qkv", bufs=2, space="PSUM") as ps_qkv,
            ):
                xt = [loads.tile([P, S], F32R, name=f"xt{i}", tag=f"xt{i}") for i in range(NE)]
                wv_s = [loads.tile([P, E], F32R, name=f"wv{i}", tag=f"wv{i}") for i in range(NE)]
                xt_r = xt_d.rearrange("(i p) s -> p i s", p=P)
                wq_r = wq_d.rearrange("(i p) e -> p i e", p=P)
                wk_r = wk_d.rearrange("(i p) e -> p i e", p=P)
                wv_r = wv_d.rearrange("(i p) e -> p i e", p=P)
                for i in range(NE):
                    nc.sync.dma_start(xt[i][:, :], xt_r[:, i, :])
                    nc.sync.dma_start(wv_s[i][:, :], wv_r[:, i, :])

                # ---- qT / kT: per pair, stationary = W slice, moving = xT ----
                for pr in range(NPAIR):
                    for w_r, b_s, dst in ((wq_r, bq_s, qt), (wk_r, bk_s, kt)):
                        wts = []
                        for i in range(NE):
                            wt = wqk_stream.tile([P, P], F32R, tag="wqk")
                            nc.sync.dma_start(wt[:, :], w_r[:, i, bass.ts(pr, P)])
                            wts.append(wt)
                        ps = ps_qkv.tile([P, S], F32, tag="ps_qk")
                        for j in range(NJ):
                            jsl = bass.ts(j, JW)
                            for i in range(NE):
                                nc.tensor.matmul(
                                    ps[:, jsl],
                                    wts[i][:, :],
                                    xt[i][:, jsl],
                                    start=(i == 0),
                                    stop=(i == NE - 1),
                                )
                        nc.vector.tensor_scalar_add(
                            dst[pr][:, :], ps[:, :], b_s[:, pr : pr + 1]
                        )

                # ---- v (natural layout): stationary = xT chunk, moving = Wv ----
                for c in range(NT):
                    ps = ps_qkv.tile([P, E], F32, tag="ps_v")
                    for n0, nw in ((0, JW), (JW, E - JW)):
                        nsl = bass.ds(n0, nw)
                        for i in range(NE):
                            nc.tensor.matmul(
                                ps[:, nsl],
                                xt[i][:, bass.ts(c, P)],
                                wv_s[i][:, nsl],
                                start=(i == 0),
                                stop=False,
                            )
                        # bias via rank-1 update: ones.T @ bv_row
                        nc.tensor.matmul(
                            ps[:, nsl],
                            ones[:, :],
                            bv_r[:, nsl],
                            start=False,
                            stop=True,
                        )
                    vx4 = vx[c][:, :].rearrange("p (h e) -> p h e", e=D + 1)
                    nc.vector.tensor_copy(
                        vx4[:, :, 0:D],
                        ps[:, :].rearrange("p (h e) -> p h e", e=D),
                    )
                    nc.sync.dma_start(
                        vx4[:, :, D],
                        ones_d[0:H].unsqueeze(0).to_broadcast((P, H)),
                    )

            # ---- attention ----
            with tc.tile_pool(name="exp_pool", bufs=EXP_BUFS) as exp_pool:
                with (
                    tc.tile_pool(name="ps_s", bufs=2, space="PSUM") as ps_s_pool,
                    tc.tile_pool(name="ps_av", bufs=2, space="PSUM") as ps_av_pool,
                ):
                    for pr in range(NPAIR):
                        exps = [[None] * NT for _ in range(2)]
                        for r in range(2):
                            rsl = bass.ds(64 * r, 64)
                            for c in range(NT):
                                et = exp_pool.tile([P, S], F32R, tag="exp")
                                for j in range(NJ):
                                    jsl = bass.ts(j, JW)
                                    ps = ps_s_pool.tile([P, JW], F32, tag="ps_s")
                                    nc.tensor.matmul(
                                        ps[:, :],
                                        kt[pr][rsl, bass.ts(c, P)],
                                        qt[pr][rsl, jsl],
                                        start=True,
                                        stop=True,
                                        tile_position=(64 * r, 0),
                                    )
                                    nc.scalar.activation(
                                        et[:, jsl],
                                        ps[:, :],
                                        mybir.ActivationFunctionType.Exp,
                                        scale=SCALE,
                                    )
                                exps[r][c] = et
                        for r in range(2):
                            h = 2 * pr + r
                            hsl = bass.ds(h * (D + 1), D + 1)
                            av = ps_av_pool.tile([P, S], F32, tag="av")
                            for c in range(NT):
                                for j in range(NJ):
                                    jsl = bass.ts(j, JW)
                                    nc.tensor.matmul(
                                        av[0 : D + 1, jsl],
                                        vx[c][:, hsl],
                                        exps[r][c][:, jsl],
                                        start=(c == 0),
                                        stop=(c == NT - 1),
                                    )
                            rec = work.tile([1, S], F32, tag="rec")
                            nc.vector.reciprocal(rec[:, :], av[D : D + 1, :])
                            bc = work.tile([D, S], F32, tag="bc")
                            nc.sync.dma_start(
                                bc[:, :],
                                rec[0:1, :].unsqueeze(1).to_broadcast((1, D, S)),
                            )
                            nc.vector.tensor_tensor(
                                at[pr][bass.ds(64 * r, 64), :],
                                av[0:D, :],
                                bc[:, :],
                                mybir.AluOpType.mult,
                            )

            # ---- output projection ----
            with tc.tile_pool(name="ps_y", bufs=2, space="PSUM") as ps_y_pool:
                for j in range(NT):
                    ps = ps_y_pool.tile([P, E], F32, tag="ps_y")
                    for n0, nw in ((0, JW), (JW, E - JW)):
                        nsl = bass.ds(n0, nw)
                        for pr in range(NPAIR):
                            nc.tensor.matmul(
                                ps[:, nsl],
                                at[pr][:, bass.ts(j, P)],
                                wo_s[pr][:, nsl],
                                start=(pr == 0),
                                stop=False,
                            )
                        nc.tensor.matmul(
                            ps[:, nsl], ones[:, :], bo_r[:, nsl], start=False, stop=True
                        )
                    ysb = work.tile([P, E], F32, tag="ysb")
                    nc.vector.tensor_copy(ysb[:, :], ps[:, :])
                    nc.sync.dma_start(y_d[bass.ts(j, P), :], ysb[:, :])

    nc.compile()
    return nc


_NC = None


def _get_nc():
    global _NC
    if _NC is None:
        _NC = build_nc()
    return _NC


def _prep_inputs(hidden_state, Wq, bq, Wk, bk, Wv, bv, Wo, bo):
    """Build the per-core input maps (data-parallel over batch)."""
    f = np.float32
    wq_p = _round_fp32r(np.asarray(Wq, dtype=f).transpose(1, 0, 2).reshape(E, E))
    wk_p = _round_fp32r(np.asarray(Wk, dtype=f).transpose(1, 0, 2).reshape(E, E))
    wv_p = _round_fp32r(np.asarray(Wv, dtype=f).transpose(1, 0, 2).reshape(E, E))
    bq_p = np.ascontiguousarray(bq.reshape(E), dtype=f)
    bk_p = np.ascontiguousarray(bk.reshape(E), dtype=f)
    bv_p = _round_fp32r(bv.reshape(E))
    wo_p = _round_fp32r(Wo)
    bo_p = _round_fp32r(bo)
    in_maps = []
    for b in range(NCORES):
        in_maps.append(
            {
                "xt": _round_fp32r(np.asarray(hidden_state[b], dtype=f).T),
                "wq": wq_p,
                "wk": wk_p,
                "wv": wv_p,
                "bq": bq_p,
                "bk": bk_p,
                "bv": bv_p,
                "wo": wo_p,
                "bo": bo_p,
                "ones": np.ones(P, dtype=f),
            }
        )
    return in_maps


def kernel(hidden_state, Wq, bq, Wk, bk, Wv, bv, Wo, bo, _trace=False):
    nc = _get_nc()
    in_maps = _prep_inputs(hidden_state, Wq, bq, Wk, bk, Wv, bv, Wo, bo)
    res = run_bass_kernel_spmd(nc, in_maps, list(range(NCORES)), trace=_trace)
    out = np.stack([np.asarray(res.results[b]["y"]) for b in range(NCORES)])
    if _trace:
        kernel.last_exec_time_ns = res.exec_time_ns
        kernel.last_res = res
    return out.astype(np.float32)


# revision 21
# speedup vs baseline: 1.5898x; 1.0909x over previous
"""Multi-head attention (B=8, S=1024, E=768, H=12, D=64) on 8 TRN2 NeuronCores.

Strategy: pure data-parallel over batch — each core processes one batch
element end-to-end (all 12 heads), so no collectives are needed. Inside a
core everything is kept "transposed" so the PE contraction dim always sits
on SBUF partitions:

  host:   xT = x[b].T                              [E, S]
  qT/kT:  out[d_pair, S]   = Wpair.T @ xT          (Wq slices stationary)
  v:      out[t, h*D]      = xT_chunk.T @ Wv_pack  (+ bias via ones-row K=1 matmul)
  scores: out[t_chunk, S]  = kT_slice.T @ qT       (K=64, two heads row-packed)
  P = exp(scale*scores)    on ACT, no max-subtraction (scores are O(1))
  AV:     out[65, S]       = [v_h | 1].T @ P       row 64 = softmax denominator
  norm:   attnT = AV[0:64] * broadcast(1/AV[64])   (DMA-broadcast + DVE mult)
  y:      y[s_chunk, E]    = attnT_pair.T @ Wo_chunk (+ bo via ones-row)
"""

import sys
import types

import numpy as np

import concourse.bacc as bacc
import concourse.bass as bass
import concourse.mybir as mybir
import concourse.tile as tile
from concourse.bass_utils import run_bass_kernel_spmd


def _ensure_ntff_hook():
    """This image's antenv lacks axon_hooks; synthesize it so trace=True works."""
    try:
        import antenv.axon_hooks  # noqa: F401

        return
    except ImportError:
        pass
    try:
        from trn_agent_boot.trn_boot import _ntff_profile_via_ctypes

        hook = _ntff_profile_via_ctypes("/opt/axon/libaxon_pjrt.so")
    except Exception:
        hook = None
    m = types.ModuleType("antenv.axon_hooks")
    m.get_axon_ntff_profile_hook = lambda: hook
    m.set_axon_ntff_profile_hook = lambda h: None
    sys.modules["antenv.axon_hooks"] = m


_ensure_ntff_hook()

F32 = mybir.dt.float32
P = 128
S = 1024
E = 768
H = 12
D = 64
NCORES = 8

NE = E // P    # 6 contraction chunks over E
NPAIR = H // 2 # 6 head pairs
NT = S // P    # 8 key/t chunks
NJ = 2         # 512-wide free-dim chunks over S
JW = 512
SCALE = 1.0 / 8.0  # 1/sqrt(D)

EXP_BUFS = 14

F32R = mybir.dt.float32r
BF16 = mybir.dt.bfloat16


import ml_dtypes


def _to_bf16(a):
    return np.ascontiguousarray(np.asarray(a, dtype=np.float32), dtype=np.float32).astype(
        ml_dtypes.bfloat16
    )


def build_nc():
    nc = bacc.Bacc(
        "TRN2",
        target_bir_lowering=False,
        debug=False,
        num_devices=NCORES,
    )
    xt_d = nc.declare_dram_parameter("xt", [E, S], BF16, isOutput=False)
    wq_d = nc.declare_dram_parameter("wq", [E, E], BF16, isOutput=False)
    wk_d = nc.declare_dram_parameter("wk", [E, E], BF16, isOutput=False)
    wv_d = nc.declare_dram_parameter("wv", [E, E], BF16, isOutput=False)
    bq_d = nc.declare_dram_parameter("bq", [E], F32, isOutput=False)
    bk_d = nc.declare_dram_parameter("bk", [E], F32, isOutput=False)
    bv_d = nc.declare_dram_parameter("bv", [E], BF16, isOutput=False)
    wo_d = nc.declare_dram_parameter("wo", [E, E], BF16, isOutput=False)
    bo_d = nc.declare_dram_parameter("bo", [E], BF16, isOutput=False)
    ones_d = nc.declare_dram_parameter("ones", [P], BF16, isOutput=False)
    y_d = nc.declare_dram_parameter("y", [S, E], F32, isOutput=True)

    with tile.TileContext(nc) as tc:
        with (
            tc.tile_pool(name="persist", bufs=1) as persist,
            tc.tile_pool(name="work", bufs=2) as work,
        ):
            # ---- persistent SBUF tensors (one producer per tile) ----
            qt = [persist.tile([P, S], BF16, name=f"qt{i}", tag=f"qt{i}") for i in range(NPAIR)]
            kt = [persist.tile([P, S], BF16, name=f"kt{i}", tag=f"kt{i}") for i in range(NPAIR)]
            vx = [
                persist.tile([P, H * (D + 1)], BF16, name=f"vx{c}", tag=f"vx{c}") for c in range(NT)
            ]
            at = [persist.tile([P, S], BF16, name=f"at{i}", tag=f"at{i}") for i in range(NPAIR)]
            wo_s = [persist.tile([P, E], BF16, name=f"wo{i}", tag=f"wo{i}") for i in range(NE)]
            bq_s = persist.tile([P, NPAIR], F32)
            bk_s = persist.tile([P, NPAIR], F32)
            bv_r = persist.tile([1, E], BF16)
            bo_r = persist.tile([1, E], BF16)
            ones = persist.tile([1, P], BF16)

            nc.sync.dma_start(ones[:, :], ones_d[:].unsqueeze(0))

            # ---- bias / weight loads ----
            nc.sync.dma_start(bq_s[:, :], bq_d[:].rearrange("(c p) -> p c", p=P))
            nc.sync.dma_start(bk_s[:, :], bk_d[:].rearrange("(c p) -> p c", p=P))
            nc.sync.dma_start(bv_r[:, :], bv_d[:].unsqueeze(0))
            nc.sync.dma_start(bo_r[:, :], bo_d[:].unsqueeze(0))
            wo_r = wo_d.rearrange("(i p) e -> p i e", p=P)
            for i in range(NE):
                nc.sync.dma_start(wo_s[i][:, :], wo_r[:, i, :])

            with (
                tc.tile_pool(name="loads", bufs=1) as loads,
                tc.tile_pool(name="wqk_stream", bufs=14) as wqk_stream,
                tc.tile_pool(name="ps_qkv", bufs=2, space="PSUM") as ps_qkv,
            ):
                xt = [loads.tile([P, S], BF16, name=f"xt{i}", tag=f"xt{i}") for i in range(NE)]
                wv_s = [loads.tile([P, E], BF16, name=f"wv{i}", tag=f"wv{i}") for i in range(NE)]
                xt_r = xt_d.rearrange("(i p) s -> p i s", p=P)
                wq_r = wq_d.rearrange("(i p) e -> p i e", p=P)
                wk_r = wk_d.rearrange("(i p) e -> p i e", p=P)
                wv_r = wv_d.rearrange("(i p) e -> p i e", p=P)
                for i in range(NE):
                    nc.sync.dma_start(xt[i][:, :], xt_r[:, i, :])
                    nc.sync.dma_start(wv_s[i][:, :], wv_r[:, i, :])

                # ---- qT / kT: per pair, stationary = W slice, moving = xT ----
                for pr in range(NPAIR):
                    for w_r, b_s, dst in ((wq_r, bq_s, qt), (wk_r, bk_s, kt)):
                        wts = []
                        for i in range(NE):
                            wt = wqk_stream.tile([P, P], BF16, tag="wqk")
                            nc.sync.dma_start(wt[:, :], w_r[:, i, bass.ts(pr, P)])
                            wts.append(wt)
                        ps = ps_qkv.tile([P, S], F32, tag="ps_qk")
                        for j in range(NJ):
                            jsl = bass.ts(j, JW)
                            for i in range(NE):
                                nc.tensor.matmul(
                                    ps[:, jsl],
                                    wts[i][:, :],
                                    xt[i][:, jsl],
                                    start=(i == 0),
                                    stop=(i == NE - 1),
                                )
                        nc.vector.tensor_scalar_add(
                            dst[pr][:, :], ps[:, :], b_s[:, pr : pr + 1]
                        )

                # ---- v (natural layout): stationary = xT chunk, moving = Wv ----
                for c in range(NT):
                    ps = ps_qkv.tile([P, E], F32, tag="ps_v")
                    for n0, nw in ((0, JW), (JW, E - JW)):
                        nsl = bass.ds(n0, nw)
                        for i in range(NE):
                            nc.tensor.matmul(
                                ps[:, nsl],
                                xt[i][:, bass.ts(c, P)],
                                wv_s[i][:, nsl],
                                start=(i == 0),
                                stop=False,
                            )
                        # bias via rank-1 update: ones.T @ bv_row
                        nc.tensor.matmul(
                            ps[:, nsl],
                            ones[:, :],
                            bv_r[:, nsl],
                            start=False,
                            stop=True,
                        )
                    vx4 = vx[c][:, :].rearrange("p (h e) -> p h e", e=D + 1)
                    nc.vector.tensor_copy(
                        vx4[:, :, 0:D],
                        ps[:, :].rearrange("p (h e) -> p h e", e=D),
                    )
                    nc.sync.dma_start(
                        vx4[:, :, D],
                        ones_d[0:H].unsqueeze(0).to_broadcast((P, H)),
                    )

            # ---- attention ----
            with tc.tile_pool(name="exp_pool", bufs=EXP_BUFS) as exp_pool:
                with (
                    tc.tile_pool(name="ps_s", bufs=2, space="PSUM") as ps_s_pool,
                    tc.tile_pool(name="ps_av", bufs=2, space="PSUM") as ps_av_pool,
                ):
                    for pr in range(NPAIR):
                        exps = [[None] * NT for _ in range(2)]
                        for r in range(2):
                            rsl = bass.ds(64 * r, 64)
                            for c in range(NT):
                                et = exp_pool.tile([P, S], BF16, tag="exp")
                                for j in range(NJ):
                                    jsl = bass.ts(j, JW)
                                    ps = ps_s_pool.tile([P, JW], F32, tag="ps_s")
                                    nc.tensor.matmul(
                                        ps[:, :],
                                        kt[pr][rsl, bass.ts(c, P)],
                                        qt[pr][rsl, jsl],
                                        start=True,
                                        stop=True,
                                        tile_position=(64 * r, 0),
                                    )
                                    nc.scalar.activation(
                                        et[:, jsl],
                                        ps[:, :],
                                        mybir.ActivationFunctionType.Exp,
                                        scale=SCALE,
                                    )
                                exps[r][c] = et
                        for r in range(2):
                            h = 2 * pr + r
                            hsl = bass.ds(h * (D + 1), D + 1)
                            av = ps_av_pool.tile([P, S], F32, tag="av")
                            for c in range(NT):
                                for j in range(NJ):
                                    jsl = bass.ts(j, JW)
                                    nc.tensor.matmul(
                                        av[0 : D + 1, jsl],
                                        vx[c][:, hsl],
                                        exps[r][c][:, jsl],
                                        start=(c == 0),
                                        stop=(c == NT - 1),
                                    )
                            rec = work.tile([1, S], F32, tag="rec")
                            nc.vector.reciprocal(rec[:, :], av[D : D + 1, :])
                            bc = work.tile([D, S], F32, tag="bc")
                            nc.sync.dma_start(
                                bc[:, :],
                                rec[0:1, :].unsqueeze(1).to_broadcast((1, D, S)),
                            )
                            nc.vector.tensor_tensor(
                                at[pr][bass.ds(64 * r, 64), :],
                                av[0:D, :],
                                bc[:, :],
                                mybir.AluOpType.mult,
                            )

            # ---- output projection ----
            with tc.tile_pool(name="ps_y", bufs=2, space="PSUM") as ps_y_pool:
                for j in range(NT):
                    ps = ps_y_pool.tile([P, E], F32, tag="ps_y")
                    for n0, nw in ((0, JW), (JW, E - JW)):
                        nsl = bass.ds(n0, nw)
                        for pr in range(NPAIR):
                            nc.tensor.matmul(
                                ps[:, nsl],
                                at[pr][:, bass.ts(j, P)],
                                wo_s[pr][:, nsl],
                                start=(pr == 0),
                                stop=False,
                            )
                        nc.tensor.matmul(
                            ps[:, nsl], ones[:, :], bo_r[:, nsl], start=False, stop=True
                        )
                    ysb = work.tile([P, E], F32, tag="ysb")
                    nc.vector.tensor_copy(ysb[:, :], ps[:, :])
                    nc.sync.dma_start(y_d[bass.ts(j, P), :], ysb[:, :])

    nc.compile()
    return nc


_NC = None


def _get_nc():
    global _NC
    if _NC is None:
        _NC = build_nc()
    return _NC


def _prep_inputs(hidden_state, Wq, bq, Wk, bk, Wv, bv, Wo, bo):
    """Build the per-core input maps (data-parallel over batch)."""
    f = np.float32
    wq_p = _to_bf16(np.asarray(Wq, dtype=f).transpose(1, 0, 2).reshape(E, E))
    wk_p = _to_bf16(np.asarray(Wk, dtype=f).transpose(1, 0, 2).reshape(E, E))
    wv_p = _to_bf16(np.asarray(Wv, dtype=f).transpose(1, 0, 2).reshape(E, E))
    bq_p = np.ascontiguousarray(bq.reshape(E), dtype=f)
    bk_p = np.ascontiguousarray(bk.reshape(E), dtype=f)
    bv_p = _to_bf16(bv.reshape(E))
    wo_p = _to_bf16(Wo)
    bo_p = _to_bf16(bo)
    in_maps = []
    for b in range(NCORES):
        in_maps.append(
            {
                "xt": _to_bf16(np.asarray(hidden_state[b], dtype=f).T),
                "wq": wq_p,
                "wk": wk_p,
                "wv": wv_p,
                "bq": bq_p,
                "bk": bk_p,
                "bv": bv_p,
                "wo": wo_p,
                "bo": bo_p,
                "ones": np.ones(P, dtype=ml_dtypes.bfloat16),
            }
        )
    return in_maps


def kernel(hidden_state, Wq, bq, Wk, bk, Wv, bv, Wo, bo, _trace=False):
    nc = _get_nc()
    in_maps = _prep_inputs(hidden_state, Wq, bq, Wk, bk, Wv, bv, Wo, bo)
    res = run_bass_kernel_spmd(nc, in_maps, list(range(NCORES)), trace=_trace)
    out = np.stack([np.asarray(res.results[b]["y"]) for b in range(NCORES)])
    if _trace:
        kernel.last_exec_time_ns = res.exec_time_ns
        kernel.last_res = res
    return out.astype(np.float32)
